# revision 5
# baseline (speedup 1.0000x reference)
"""Trainium2 Bass kernel for batch-8 multi-head attention (B=8, N=1024, C=768, H=12).

Distribution: pure data parallelism — batch element i runs entirely on core i
(weights replicated, zero collectives).

Per-core layout strategy (everything [channel-on-partition, token-on-free]):
  xT[k, t]            via PE transposes of the DMA'd x tiles
  qT/kT[c, t]         = W_qkv chunk (stationary) x xT (moving)      [transposed]
  v[t, c]             = xT chunk (stationary) x W_v (moving)        [natural]
  ST[s, t]            = kT slice (stationary) x qT (moving)          (scores^T)
  expST               = ACT exp(SCALE * ST), PSUM->SBUF bf16
  y65[d|sum, t]       = [v|ones] (stationary) x expST (moving): row 64 = colsum
  yT[d, t]            = y65[0:64] * broadcast(1/colsum)   (PE bcast via fp32r)
  z[t, c]             = yT slice (stationary) x W_proj (moving) + b  [natural]

All matmuls bf16 x bf16 -> fp32 PSUM, 512-wide halves (PSUM bank limit).
"""
import numpy as np

import concourse.bacc as bacc
import concourse.bass as bass
import concourse.tile as tile
import concourse.mybir as mybir
from concourse import masks
from concourse.bass_utils import run_bass_kernel_spmd

F32 = mybir.dt.float32
F32R = mybir.dt.float32r
BF16 = mybir.dt.bfloat16

B, N, C = 8, 1024, 768
H, D = 12, 64
SCALE = float(D) ** -0.5
N_CORES = 8
KT = C // 128            # 6 contraction chunks of 128
TT = N // 128            # 8 token tiles of 128
ST = N // 128            # 8 key tiles of 128
CO_QK = 2 * C // 128     # 12 output chunks for q,k (transposed layout)
EXP_FN = mybir.ActivationFunctionType.Exp


def build_nc():
    nc = bacc.Bacc("TRN2", target_bir_lowering=False, debug=False,
                   num_devices=N_CORES)
    x_ext = nc.dram_tensor("x", [N, C], F32, kind="ExternalInput")
    wqkv_ext = nc.dram_tensor("W_qkv", [C, 3 * C], F32, kind="ExternalInput")
    wproj_ext = nc.dram_tensor("W_proj", [C, C], F32, kind="ExternalInput")
    bproj_ext = nc.dram_tensor("b_proj", [C], F32, kind="ExternalInput")
    out_ext = nc.dram_tensor("out", [N, C], F32, kind="ExternalOutput")

    with tile.TileContext(nc) as tc:
        with (
            tc.tile_pool(name="const", bufs=1) as constp,
            tc.tile_pool(name="wq", bufs=1) as wqp,
            tc.tile_pool(name="wstage", bufs=2) as wstage,
            tc.tile_pool(name="xstage", bufs=2) as xstage,
            tc.tile_pool(name="xt", bufs=1) as xtp,
            tc.tile_pool(name="qk", bufs=1) as qkp,
            tc.tile_pool(name="vp", bufs=1) as vp,
            tc.tile_pool(name="yt", bufs=1) as ytp,
            tc.tile_pool(name="exp", bufs=12) as expp,
            tc.tile_pool(name="recip", bufs=2) as recipp,
            tc.tile_pool(name="z", bufs=2) as zp,
            tc.tile_pool(name="psum", bufs=4, space="PSUM") as psum,
        ):
            # ---- constants ----
            ident = constp.tile([128, 128], BF16)
            masks.make_identity(nc, ident[:])
            ones_bf = constp.tile([128, 128], BF16)
            nc.gpsimd.memset(ones_bf[:], 1.0)
            b_sb = constp.tile([1, C], F32)
            nc.sync.dma_start(b_sb[:], bproj_ext[:].rearrange("(a c) -> a c", a=1))
            b_bf = constp.tile([1, C], BF16)
            nc.vector.tensor_copy(b_bf[:], b_sb[:])
            b_bcast = constp.tile([128, C], F32)

            # ---- persistent tensors ----
            xt_bf = xtp.tile([128, KT * N], BF16)          # xT: chunk k at cols [k*N, (k+1)*N)
            wq_bf = wqp.tile([128, KT * 3 * C], BF16)      # W_qkv chunk k at cols [k*3C, ...)
            wp_bf = wqp.tile([128, KT * C], BF16)          # W_proj chunk k at cols [k*C, ...)
            qk_bf = qkp.tile([128, CO_QK * N], BF16)       # qT,kT: chunk co at cols [co*N, ...)
            v65 = vp.tile([128, ST * H * 65], BF16)        # per s-tile: H blocks of [v_h(64)|1]
            yt_bf = ytp.tile([128, KT * N], BF16)          # yT: chunk c at cols [c*N, ...)

            def halves(width):
                out = []
                off = 0
                while off < width:
                    w = min(512, width - off)
                    out.append((off, w))
                    off += w
                return out

            # ---- phase 1: load x, convert, transpose ----
            for t in range(TT):
                x_f = xstage.tile([128, C], F32, tag="xf")
                nc.sync.dma_start(x_f[:], x_ext[t * 128:(t + 1) * 128, :])
                x_bf = xstage.tile([128, C], BF16, tag="xbf")
                nc.vector.tensor_copy(x_bf[:], x_f[:])
                for k in range(KT):
                    tp_ps = psum.tile([128, 128], BF16, tag="ps")
                    nc.tensor.transpose(tp_ps[:], x_bf[:, k * 128:(k + 1) * 128], ident[:])
                    dst = xt_bf[:, k * N + t * 128: k * N + (t + 1) * 128]
                    if k % 2 == 0:
                        nc.vector.tensor_copy(dst, tp_ps[:])
                    else:
                        nc.scalar.copy(dst, tp_ps[:])

            # ---- phase 2: load weights, convert ----
            for k in range(KT):
                w_f = wstage.tile([128, 3 * C], F32, tag="wf")
                nc.sync.dma_start(w_f[:], wqkv_ext[k * 128:(k + 1) * 128, :])
                nc.gpsimd.tensor_copy(wq_bf[:, k * 3 * C:(k + 1) * 3 * C], w_f[:])
            for k in range(KT):
                w_f = wstage.tile([128, 3 * C], F32, tag="wf")
                nc.sync.dma_start(w_f[:, 0:C], wproj_ext[k * 128:(k + 1) * 128, :])
                nc.gpsimd.tensor_copy(wp_bf[:, k * C:(k + 1) * C], w_f[:, 0:C])

            # b broadcast to 128 partitions via bf16 matmul
            b_ps = psum.tile([128, C], F32, tag="ps")
            for off, w in halves(C):
                nc.tensor.matmul(b_ps[:, off:off + w], ones_bf[0:1, 0:128],
                                 b_bf[0:1, off:off + w], start=True, stop=True)
            nc.scalar.copy(b_bcast[:], b_ps[:])

            # ---- phase 3: qT, kT = (x @ W_{q,k})^T ----
            for co in range(CO_QK):
                qk_ps = psum.tile([128, N], F32, tag="ps")
                for k in range(KT):
                    lhsT = wq_bf[:, k * 3 * C + co * 128: k * 3 * C + (co + 1) * 128]
                    for off, w in halves(N):
                        nc.tensor.matmul(qk_ps[:, off:off + w], lhsT,
                                         xt_bf[:, k * N + off: k * N + off + w],
                                         start=(k == 0), stop=(k == KT - 1))
                nc.vector.tensor_copy(qk_bf[:, co * N:(co + 1) * N], qk_ps[:])

            # ---- phase 4: v natural layout, interleaved [v_h | 1] blocks ----
            for t in range(TT):
                v_ps = psum.tile([128, C], F32, tag="ps")
                for k in range(KT):
                    lhsT = xt_bf[:, k * N + t * 128: k * N + (t + 1) * 128]
                    for off, w in halves(C):
                        nc.tensor.matmul(v_ps[:, off:off + w], lhsT,
                                         wq_bf[:, k * 3 * C + 2 * C + off: k * 3 * C + 2 * C + off + w],
                                         start=(k == 0), stop=(k == KT - 1))
                base = t * H * 65
                v_view = v65[:, base: base + H * 65].rearrange("p (h w) -> p h w", w=65)
                nc.vector.tensor_copy(v_view[:, :, 0:64],
                                      v_ps[:].rearrange("p (h d) -> p h d", d=64))
                nc.gpsimd.memset(v_view[:, :, 64:65], 1.0)

            # ---- phase 5: attention per head ----
            for h in range(H):
                po = (h % 2) * 64
                q_ap = qk_bf[po:po + 64, (h // 2) * N:(h // 2 + 1) * N]
                k_ap = qk_bf[po:po + 64, (6 + h // 2) * N:(6 + h // 2 + 1) * N]
                e_tiles = []
                for s in range(ST):
                    s_ps = psum.tile([128, N], F32, tag="ps")
                    for off, w in halves(N):
                        nc.tensor.matmul(s_ps[:, off:off + w],
                                         k_ap[:, s * 128:(s + 1) * 128],
                                         q_ap[:, off:off + w],
                                         start=True, stop=True)
                    e_t = expp.tile([128, N], BF16, tag="exp")
                    nc.scalar.activation(e_t[:], s_ps[:], EXP_FN, bias=0.0, scale=SCALE)
                    e_tiles.append(e_t)
                y_ps = psum.tile([128, N], F32, tag="ps")
                for s in range(ST):
                    lhsT = v65[:, s * H * 65 + h * 65: s * H * 65 + (h + 1) * 65]
                    for off, w in halves(N):
                        nc.tensor.matmul(y_ps[0:65, off:off + w], lhsT,
                                         e_tiles[s][:, off:off + w],
                                         start=(s == 0), stop=(s == ST - 1))
                recip = recipp.tile([1, N], F32, tag="recip")
                nc.vector.reciprocal(recip[0:1, :], y_ps[64:65, :])
                r_sb = recipp.tile([64, N], F32, tag="rbc")
                nc.gpsimd.partition_broadcast(r_sb[0:64, :], recip[0:1, :], channels=64)
                dst = yt_bf[po:po + 64, (h // 2) * N:(h // 2 + 1) * N]
                nc.vector.tensor_mul(dst, y_ps[0:64, :], r_sb[0:64, :])

            # ---- phase 6: out = yT^T @ W_proj + b ----
            for t in range(TT):
                z_ps = psum.tile([128, C], F32, tag="ps")
                for k in range(KT):
                    lhsT = yt_bf[:, k * N + t * 128: k * N + (t + 1) * 128]
                    for off, w in halves(C):
                        nc.tensor.matmul(z_ps[:, off:off + w], lhsT,
                                         wp_bf[:, k * C + off: k * C + off + w],
                                         start=(k == 0), stop=(k == KT - 1))
                z_sb = zp.tile([128, C], F32, tag="z")
                nc.vector.tensor_add(z_sb[:], z_ps[:], b_bcast[:])
                nc.sync.dma_start(out_ext[t * 128:(t + 1) * 128, :], z_sb[:])

    nc.finalize()
    return nc


_NC = None


def _get_nc():
    global _NC
    if _NC is None:
        _NC = build_nc()
    return _NC


def _run(x, W_qkv, W_proj, b_proj, trace=False):
    nc = _get_nc()
    W_qkv = np.ascontiguousarray(W_qkv, dtype=np.float32)
    W_proj = np.ascontiguousarray(W_proj, dtype=np.float32)
    b_proj = np.ascontiguousarray(b_proj, dtype=np.float32)
    in_maps = [
        {
            "x": np.ascontiguousarray(x[i], dtype=np.float32),
            "W_qkv": W_qkv,
            "W_proj": W_proj,
            "b_proj": b_proj,
        }
        for i in range(N_CORES)
    ]
    res = run_bass_kernel_spmd(nc, in_maps, core_ids=list(range(N_CORES)),
                               trace=trace)
    out = np.stack([res.results[i]["out"] for i in range(N_CORES)], axis=0)
    return out.astype(np.float32), res


def kernel(x, W_qkv, W_proj, b_proj):
    out, _ = _run(x, W_qkv, W_proj, b_proj, trace=False)
    return out
